# revision 22
# baseline (speedup 1.0000x reference)
"""Trainium2 kernel for CustomWaveletLayer.

Math: out[b,o] = sum_{i,w} coef[o,i,w] * morlet(tanh(x[b,i]*tanh_range)*zoom[o,i,w] - pan[o,i,w])
with morlet(z) = cos(5z)*exp(-z^2/2).

Key identity: out[b,o] = sum_i G_oi(t[b,i]) with t = tanh(x*tanh_range) in (-1,1),
G_oi smooth 1-D functions. Host expands each G_oi in a 14-function dictionary
(T_0..T_10 Chebyshev + 3 Gaussians at mu = 0, +-MU) by ridge least squares;
device evaluates the dictionary and contracts with the coefficients:

    out[b,o] = sum_k sum_i V_k(t[b,i]) * C[k,o,i]

On-device per core (128-row batch shard):
  ACT: tanh, then the 3 Gaussians (Square+Exp share tanh's table set -> one load)
  DVE: paired Chebyshev recurrence P_j = [T_{2j+1}|T_{2j+2}],
       P_j = [beta|beta] * P_{j-1} - P_{j-2} with beta = 2*T_2 (halves op count)
  PE:  14 PSUM-accumulated fp16 128x128x128 matmuls, coefficients stationary.
Output computed transposed [o,b]; host transposes back. Data-parallel over
batch on 8 cores.
"""

import numpy as np

import concourse.bass as bass
import concourse.mybir as mybir
from concourse import bacc, bass_utils
from concourse.tile import TileContext

B, I, O, W = 1024, 128, 128, 8
NCORES = 8
BS = B // NCORES  # batch shard per core
NCHEB = 9
# 1-op ACT units fn(a*(t-c)): ('dgelu'|'tanh', a, c), fitted offline
UNITS = (
    ("dgelu", 4.3241, 0.3049),
    ("dgelu", 3.2294, 0.3377),
    ("dgelu", 2.9293, -0.2819),
    ("tanh", 6.5907, 0.4657),
    ("tanh", 2.7533, -0.8465),
)
K = NCHEB + len(UNITS)  # 14 basis functions

_F32 = mybir.dt.float32
_F16 = mybir.dt.float16

_nc_cache = {}


def _build_nc(k_terms: int) -> bass.Bass:
    """k_terms selects the variant: K -> mixed dictionary, otherwise a pure
    Chebyshev fallback of k_terms terms (generic-input insurance)."""
    if k_terms in _nc_cache:
        return _nc_cache[k_terms]
    mixed = k_terms == K
    kA = 7  # weight chunk split for parallel DMA
    nc = bacc.Bacc()
    xt = nc.dram_tensor("xt", [I, BS], _F16, kind="ExternalInput")  # [i, b] pre-scaled
    cw = nc.dram_tensor("cw", [I, k_terms * O], _F16, kind="ExternalInput")  # [i,(k,o)]
    out = nc.dram_tensor("out", [O, BS], _F32, kind="ExternalOutput")  # [o, b]

    AF = mybir.ActivationFunctionType
    with TileContext(nc) as tc:
        with (
            tc.tile_pool(name="io", bufs=2) as io_pool,
            tc.tile_pool(name="w", bufs=2) as w_pool,
            tc.tile_pool(name="v", bufs=k_terms + 6) as v_pool,
            tc.tile_pool(name="ps", bufs=1, space="PSUM") as ps_pool,
        ):
            # input halves on two queues so tanh starts right after the
            # ACT table load; weight chunks follow on the same queues
            xs = io_pool.tile([I, BS], _F16, tag="xs")
            nc.sync.dma_start(xs[:44, :], xt[:44, :])
            nc.scalar.dma_start(xs[44:88, :], xt[44:88, :])
            nc.gpsimd.dma_start(xs[88:, :], xt[88:, :])
            wsA = w_pool.tile([I, kA * O], _F16, tag="wA")
            nc.sync.dma_start(wsA[:], cw[:, : kA * O])
            wsB = w_pool.tile([I, (k_terms - kA) * O], _F16, tag="wB")
            nc.gpsimd.dma_start(wsB[:], cw[:, kA * O :])

            # dummy activation on an always-ready tile: hoists the ACT
            # table load so it overlaps the input DMA instead of following it
            warm = io_pool.tile([I, 1], _F16, tag="warm")
            nc.vector.memset(warm[:], 0.0)
            warm2 = io_pool.tile([I, 1], _F16, tag="warm")
            nc.scalar.activation(warm2[:], warm[:], AF.Tanh)

            def wslice(k):
                if k < kA:
                    return wsA[:, k * O : (k + 1) * O]
                return wsB[:, (k - kA) * O : (k - kA + 1) * O]

            t = v_pool.tile([I, BS], _F16, tag="t")
            nc.scalar.activation(t[:], xs[:], AF.Tanh)

            V = [None] * k_terms  # basis tiles (APs) in coefficient order
            ones = v_pool.tile([I, BS], _F16, tag="ones")
            nc.vector.memset(ones[:], 1.0)
            V[0] = ones[:]
            V[1] = t[:]

            if mixed:
                # ACT: 1-op units fn(a*t - a*c); tanh/gelu/dgelu share a table set
                fnmap = {"dgelu": AF.Derivative_Gelu, "tanh": AF.Tanh}
                for j, (fname, ua, uc) in enumerate(UNITS):
                    bt = v_pool.tile([I, 1], _F32, tag="bias")
                    nc.vector.memset(bt[:], -ua * uc)
                    g = v_pool.tile([I, BS], _F16, tag="g")
                    nc.scalar.activation(g[:], t[:], fnmap[fname], scale=ua,
                                         bias=bt[:])
                    V[NCHEB + j] = g[:]

                # Chebyshev composition tree split across DVE and GpSimd:
                #   T_{2k} = 2*T_k^2 - 1   (squares -> GpSimd)
                #   T_{m+1} via 2*T_m*T_{m?}-t fused with scalar_tensor_tensor
                MULT, ADD, SUB = (mybir.AluOpType.mult, mybir.AluOpType.add,
                                  mybir.AluOpType.subtract)

                def tile16(tag):
                    return v_pool.tile([I, BS], _F16, name=tag, tag=tag)

                s = tile16("s")
                nc.vector.tensor_mul(s[:], t[:], t[:])          # t^2        DVE d1
                T2 = tile16("v")
                nc.vector.tensor_scalar(T2[:], s[:], 2.0, -1.0, MULT, ADD)  # d2
                w3 = tile16("w3")
                nc.vector.tensor_scalar(w3[:], s[:], 4.0, -3.0, MULT, ADD)  # d2
                T3 = tile16("v")
                nc.vector.tensor_mul(T3[:], t[:], w3[:])        #            d3
                s4 = tile16("sq")
                nc.gpsimd.tensor_mul(s4[:], T2[:], T2[:])       # T2^2  POOL d3
                T4 = tile16("v")
                nc.gpsimd.tensor_scalar(T4[:], s4[:], 2.0, -1.0, MULT, ADD)
                m5 = tile16("m")
                nc.vector.tensor_mul(m5[:], T2[:], T3[:])       #            d4
                T5 = tile16("v")
                nc.vector.scalar_tensor_tensor(T5[:], m5[:], 2.0, t[:], MULT, SUB)
                s6 = tile16("sq")
                nc.gpsimd.tensor_mul(s6[:], T3[:], T3[:])       # T3^2  POOL d4
                T6 = tile16("v")
                nc.gpsimd.tensor_scalar(T6[:], s6[:], 2.0, -1.0, MULT, ADD)
                m7 = tile16("m")
                nc.vector.tensor_mul(m7[:], T3[:], T4[:])       #            d5
                T7 = tile16("v")
                nc.vector.scalar_tensor_tensor(T7[:], m7[:], 2.0, t[:], MULT, SUB)
                s8 = tile16("sq")
                nc.gpsimd.tensor_mul(s8[:], T4[:], T4[:])       # T4^2  POOL d5
                T8 = tile16("v")
                nc.gpsimd.tensor_scalar(T8[:], s8[:], 2.0, -1.0, MULT, ADD)
                for idx, tl in zip(range(2, 9), (T2, T3, T4, T5, T6, T7, T8)):
                    V[idx] = tl[:]
                # matmul issue order ~ readiness order (units land ~mid-chain)
                order = [0, 1, 2, 3, 9, 10, 4, 11, 6, 12, 13, 5, 7, 8]
            else:
                u = v_pool.tile([I, BS], _F16, tag="u")
                nc.vector.tensor_scalar_mul(u[:], t[:], 2.0)
                for k in range(2, k_terms):
                    p = v_pool.tile([I, BS], _F16, tag="p")
                    nc.vector.tensor_mul(p[:], u[:], V[k - 1])
                    vk = v_pool.tile([I, BS], _F16, tag="v")
                    nc.vector.tensor_sub(vk[:], p[:], V[k - 2])
                    V[k] = vk[:]
                order = list(range(k_terms))

            acc = ps_pool.tile([O, BS], _F32)
            for n, k in enumerate(order):
                nc.tensor.matmul(
                    acc[:], wslice(k), V[k],
                    start=(n == 0), stop=(n == k_terms - 1),
                )

            res = io_pool.tile([O, BS], _F32, tag="res")
            nc.vector.tensor_copy(res[:64, :], acc[:64, :])
            nc.scalar.copy(res[64:, :], acc[64:, :])
            nc.sync.dma_start(out[:64, :], res[:64, :])
            nc.scalar.dma_start(out[64:96, :], res[64:96, :])
            nc.gpsimd.dma_start(out[96:, :], res[96:, :])

    nc.compile()  # bacc passes: wait splitting, reg alloc, act table loads
    _nc_cache[k_terms] = nc
    return nc


_erf = np.vectorize(__import__("math").erf)


def _unit_fn(fname, z):
    if fname == "tanh":
        return np.tanh(z)
    # derivative of exact gelu: Phi(z) + z*phi(z)
    phi = np.exp(-0.5 * z * z) / np.sqrt(2 * np.pi)
    cdf = 0.5 * (1.0 + _erf(z / np.sqrt(2.0)))
    return cdf + z * phi


def _dict_mat(q, k_terms):
    mixed = k_terms == K
    ncheb = NCHEB if mixed else k_terms
    v = np.empty((len(q), k_terms))
    v[:, 0] = 1.0
    v[:, 1] = q
    for k in range(2, ncheb):
        v[:, k] = 2.0 * q * v[:, k - 1] - v[:, k - 2]
    if mixed:
        for j, (fname, ua, uc) in enumerate(UNITS):
            v[:, NCHEB + j] = _unit_fn(fname, ua * (q - uc))
    return v


def _fit(coef, zoom, pan, k_terms, quad=129):
    """Project G_oi(t) = sum_w coef*morlet(t*zoom-pan) onto the dictionary by
    (ridge) least squares on a Lobatto grid. Returns fp16 [i, (k,o)] slab."""
    q = np.cos(np.pi * np.arange(quad) / (quad - 1))
    z = q[:, None, None, None] * zoom[None] - pan[None]
    m = (np.cos(5.0 * z) * np.exp(-0.5 * z * z) * coef[None]).sum(-1)  # [Q, O, I]
    a = _dict_mat(q, k_terms)
    sol = np.linalg.solve(a.T @ a + 1e-8 * np.eye(k_terms), a.T @ m.reshape(quad, -1))
    resid = np.abs(a @ sol - m.reshape(quad, -1)).max()
    coefmax = np.abs(sol).max()
    ck = sol.reshape(k_terms, m.shape[1], m.shape[2]).transpose(2, 0, 1)  # [i, k, o]
    return np.ascontiguousarray(ck.reshape(ck.shape[0], -1), np.float16), resid, coefmax


def kernel(x, tanh_range, coef, zoom, pan):
    x = np.asarray(x, np.float32)
    coef = np.asarray(coef, np.float32)
    zoom = np.asarray(zoom, np.float32)
    pan = np.asarray(pan, np.float32)
    tr = float(np.asarray(tanh_range))

    k_terms = K
    ck, resid, coefmax = _fit(coef, zoom, pan, k_terms)
    if resid > 2e-4 or coefmax > 4.0:  # insurance for atypical inputs
        k_terms = 24
        ck, resid, coefmax = _fit(coef, zoom, pan, k_terms)

    xt = np.ascontiguousarray((x * tr).T, np.float16)  # [I, B]

    in_maps = [
        {"xt": np.ascontiguousarray(xt[:, c * BS : (c + 1) * BS]), "cw": ck}
        for c in range(NCORES)
    ]
    nc = _build_nc(k_terms)
    res = bass_utils.run_bass_kernel_spmd(nc, in_maps, core_ids=list(range(NCORES)))
    return np.concatenate([r["out"].T for r in res.results], axis=0)


# revision 23
# speedup vs baseline: 1.0557x; 1.0557x over previous
"""Trainium2 kernel for CustomWaveletLayer.

Math: out[b,o] = sum_{i,w} coef[o,i,w] * morlet(tanh(x[b,i]*tanh_range)*zoom[o,i,w] - pan[o,i,w])
with morlet(z) = cos(5z)*exp(-z^2/2).

Key identity: out[b,o] = sum_i G_oi(t[b,i]) with t = tanh(x*tanh_range) in (-1,1),
G_oi smooth 1-D functions. Host expands each G_oi in a 14-function dictionary
(T_0..T_10 Chebyshev + 3 Gaussians at mu = 0, +-MU) by ridge least squares;
device evaluates the dictionary and contracts with the coefficients:

    out[b,o] = sum_k sum_i V_k(t[b,i]) * C[k,o,i]

On-device per core (128-row batch shard):
  ACT: tanh, then the 3 Gaussians (Square+Exp share tanh's table set -> one load)
  DVE: paired Chebyshev recurrence P_j = [T_{2j+1}|T_{2j+2}],
       P_j = [beta|beta] * P_{j-1} - P_{j-2} with beta = 2*T_2 (halves op count)
  PE:  14 PSUM-accumulated fp16 128x128x128 matmuls, coefficients stationary.
Output computed transposed [o,b]; host transposes back. Data-parallel over
batch on 8 cores.
"""

import numpy as np

import concourse.bass as bass
import concourse.mybir as mybir
from concourse import bacc, bass_utils
from concourse.tile import TileContext

B, I, O, W = 1024, 128, 128, 8
NCORES = 8
BS = B // NCORES  # batch shard per core
NCHEB = 9
# 1-op ACT units fn(a*(t-c)): ('dgelu'|'tanh', a, c), fitted offline
UNITS = (
    ("dgelu", 4.3241, 0.3049),
    ("dgelu", 3.2294, 0.3377),
    ("dgelu", 2.9293, -0.2819),
    ("tanh", 6.5907, 0.4657),
    ("tanh", 2.7533, -0.8465),
)
K = NCHEB + len(UNITS)  # 14 basis functions

_F32 = mybir.dt.float32
_F16 = mybir.dt.float16

_nc_cache = {}


def _build_nc(k_terms: int) -> bass.Bass:
    """k_terms selects the variant: K -> mixed dictionary, otherwise a pure
    Chebyshev fallback of k_terms terms (generic-input insurance)."""
    if k_terms in _nc_cache:
        return _nc_cache[k_terms]
    mixed = k_terms == K
    kA = 7  # weight chunk split for parallel DMA
    nc = bacc.Bacc()
    xt = nc.dram_tensor("xt", [I, BS], _F16, kind="ExternalInput")  # [i, b] pre-scaled
    cw = nc.dram_tensor("cw", [I, k_terms * O], _F16, kind="ExternalInput")  # [i,(k,o)]
    out = nc.dram_tensor("out", [O, BS], _F32, kind="ExternalOutput")  # [o, b]

    AF = mybir.ActivationFunctionType
    with TileContext(nc) as tc:
        with (
            tc.tile_pool(name="io", bufs=2) as io_pool,
            tc.tile_pool(name="w", bufs=2) as w_pool,
            tc.tile_pool(name="v", bufs=k_terms + 6) as v_pool,
            tc.tile_pool(name="ps", bufs=1, space="PSUM") as ps_pool,
        ):
            # input halves on two queues so tanh starts right after the
            # ACT table load; weight chunks follow on the same queues
            xs = io_pool.tile([I, BS], _F16, tag="xs")
            nc.sync.dma_start(xs[:64, :], xt[:64, :])
            nc.scalar.dma_start(xs[64:, :], xt[64:, :])
            wsA = w_pool.tile([I, kA * O], _F16, tag="wA")
            nc.sync.dma_start(wsA[:], cw[:, : kA * O])
            wsB = w_pool.tile([I, (k_terms - kA) * O], _F16, tag="wB")
            nc.gpsimd.dma_start(wsB[:], cw[:, kA * O :])

            # dummy activation on an always-ready tile: hoists the ACT
            # table load so it overlaps the input DMA instead of following it
            warm = io_pool.tile([I, 1], _F16, tag="warm")
            nc.vector.memset(warm[:], 0.0)
            warm2 = io_pool.tile([I, 1], _F16, tag="warm")
            nc.scalar.activation(warm2[:], warm[:],
                                 AF.Derivative_Gelu if mixed else AF.Tanh)

            def wslice(k):
                if k < kA:
                    return wsA[:, k * O : (k + 1) * O]
                return wsB[:, (k - kA) * O : (k - kA + 1) * O]

            t = v_pool.tile([I, BS], _F16, tag="t")
            nc.scalar.activation(t[:], xs[:], AF.Tanh)

            V = [None] * k_terms  # basis tiles (APs) in coefficient order
            ones = v_pool.tile([I, BS], _F16, tag="ones")
            nc.vector.memset(ones[:], 1.0)
            V[0] = ones[:]
            V[1] = t[:]

            if mixed:
                # ACT: 1-op units fn(a*t - a*c); tanh/gelu/dgelu share a table set
                fnmap = {"dgelu": AF.Derivative_Gelu, "tanh": AF.Tanh}
                for j, (fname, ua, uc) in enumerate(UNITS):
                    bt = v_pool.tile([I, 1], _F32, tag="bias")
                    nc.vector.memset(bt[:], -ua * uc)
                    g = v_pool.tile([I, BS], _F16, tag="g")
                    nc.scalar.activation(g[:], t[:], fnmap[fname], scale=ua,
                                         bias=bt[:])
                    V[NCHEB + j] = g[:]

                # Chebyshev composition tree split across DVE and GpSimd:
                #   T_{2k} = 2*T_k^2 - 1   (squares -> GpSimd)
                #   T_{m+1} via 2*T_m*T_{m?}-t fused with scalar_tensor_tensor
                MULT, ADD, SUB = (mybir.AluOpType.mult, mybir.AluOpType.add,
                                  mybir.AluOpType.subtract)

                def tile16(tag):
                    return v_pool.tile([I, BS], _F16, name=tag, tag=tag)

                s = tile16("s")
                nc.vector.tensor_mul(s[:], t[:], t[:])          # t^2        DVE d1
                T2 = tile16("v")
                nc.vector.tensor_scalar(T2[:], s[:], 2.0, -1.0, MULT, ADD)  # d2
                w3 = tile16("w3")
                nc.vector.tensor_scalar(w3[:], s[:], 4.0, -3.0, MULT, ADD)  # d2
                T3 = tile16("v")
                nc.vector.tensor_mul(T3[:], t[:], w3[:])        #            d3
                s4 = tile16("sq")
                nc.gpsimd.tensor_mul(s4[:], T2[:], T2[:])       # T2^2  POOL d3
                T4 = tile16("v")
                nc.gpsimd.tensor_scalar(T4[:], s4[:], 2.0, -1.0, MULT, ADD)
                m5 = tile16("m")
                nc.vector.tensor_mul(m5[:], T2[:], T3[:])       #            d4
                T5 = tile16("v")
                nc.vector.scalar_tensor_tensor(T5[:], m5[:], 2.0, t[:], MULT, SUB)
                s6 = tile16("sq")
                nc.gpsimd.tensor_mul(s6[:], T3[:], T3[:])       # T3^2  POOL d4
                T6 = tile16("v")
                nc.gpsimd.tensor_scalar(T6[:], s6[:], 2.0, -1.0, MULT, ADD)
                m7 = tile16("m")
                nc.vector.tensor_mul(m7[:], T3[:], T4[:])       #            d5
                T7 = tile16("v")
                nc.vector.scalar_tensor_tensor(T7[:], m7[:], 2.0, t[:], MULT, SUB)
                s8 = tile16("sq")
                nc.gpsimd.tensor_mul(s8[:], T4[:], T4[:])       # T4^2  POOL d5
                T8 = tile16("v")
                nc.gpsimd.tensor_scalar(T8[:], s8[:], 2.0, -1.0, MULT, ADD)
                for idx, tl in zip(range(2, 9), (T2, T3, T4, T5, T6, T7, T8)):
                    V[idx] = tl[:]
                # matmul issue order ~ readiness order (units land ~mid-chain)
                order = [0, 1, 2, 3, 9, 10, 4, 11, 6, 12, 13, 5, 7, 8]
            else:
                u = v_pool.tile([I, BS], _F16, tag="u")
                nc.vector.tensor_scalar_mul(u[:], t[:], 2.0)
                for k in range(2, k_terms):
                    p = v_pool.tile([I, BS], _F16, tag="p")
                    nc.vector.tensor_mul(p[:], u[:], V[k - 1])
                    vk = v_pool.tile([I, BS], _F16, tag="v")
                    nc.vector.tensor_sub(vk[:], p[:], V[k - 2])
                    V[k] = vk[:]
                order = list(range(k_terms))

            acc = ps_pool.tile([O, BS], _F32)
            for n, k in enumerate(order):
                nc.tensor.matmul(
                    acc[:], wslice(k), V[k],
                    start=(n == 0), stop=(n == k_terms - 1),
                )

            res = io_pool.tile([O, BS], _F32, tag="res")
            nc.vector.tensor_copy(res[:64, :], acc[:64, :])
            nc.scalar.copy(res[64:, :], acc[64:, :])
            nc.sync.dma_start(out[:64, :], res[:64, :])
            nc.scalar.dma_start(out[64:, :], res[64:, :])

    nc.compile()  # bacc passes: wait splitting, reg alloc, act table loads
    _nc_cache[k_terms] = nc
    return nc


_erf = np.vectorize(__import__("math").erf)


def _unit_fn(fname, z):
    if fname == "tanh":
        return np.tanh(z)
    # derivative of exact gelu: Phi(z) + z*phi(z)
    phi = np.exp(-0.5 * z * z) / np.sqrt(2 * np.pi)
    cdf = 0.5 * (1.0 + _erf(z / np.sqrt(2.0)))
    return cdf + z * phi


def _dict_mat(q, k_terms):
    mixed = k_terms == K
    ncheb = NCHEB if mixed else k_terms
    v = np.empty((len(q), k_terms))
    v[:, 0] = 1.0
    v[:, 1] = q
    for k in range(2, ncheb):
        v[:, k] = 2.0 * q * v[:, k - 1] - v[:, k - 2]
    if mixed:
        for j, (fname, ua, uc) in enumerate(UNITS):
            v[:, NCHEB + j] = _unit_fn(fname, ua * (q - uc))
    return v


def _fit(coef, zoom, pan, k_terms, quad=129):
    """Project G_oi(t) = sum_w coef*morlet(t*zoom-pan) onto the dictionary by
    (ridge) least squares on a Lobatto grid. Returns fp16 [i, (k,o)] slab."""
    q = np.cos(np.pi * np.arange(quad) / (quad - 1))
    z = q[:, None, None, None] * zoom[None] - pan[None]
    m = (np.cos(5.0 * z) * np.exp(-0.5 * z * z) * coef[None]).sum(-1)  # [Q, O, I]
    a = _dict_mat(q, k_terms)
    sol = np.linalg.solve(a.T @ a + 1e-8 * np.eye(k_terms), a.T @ m.reshape(quad, -1))
    resid = np.abs(a @ sol - m.reshape(quad, -1)).max()
    coefmax = np.abs(sol).max()
    ck = sol.reshape(k_terms, m.shape[1], m.shape[2]).transpose(2, 0, 1)  # [i, k, o]
    return np.ascontiguousarray(ck.reshape(ck.shape[0], -1), np.float16), resid, coefmax


def kernel(x, tanh_range, coef, zoom, pan):
    x = np.asarray(x, np.float32)
    coef = np.asarray(coef, np.float32)
    zoom = np.asarray(zoom, np.float32)
    pan = np.asarray(pan, np.float32)
    tr = float(np.asarray(tanh_range))

    k_terms = K
    ck, resid, coefmax = _fit(coef, zoom, pan, k_terms)
    if resid > 2e-4 or coefmax > 4.0:  # insurance for atypical inputs
        k_terms = 24
        ck, resid, coefmax = _fit(coef, zoom, pan, k_terms)

    xt = np.ascontiguousarray((x * tr).T, np.float16)  # [I, B]

    in_maps = [
        {"xt": np.ascontiguousarray(xt[:, c * BS : (c + 1) * BS]), "cw": ck}
        for c in range(NCORES)
    ]
    nc = _build_nc(k_terms)
    res = bass_utils.run_bass_kernel_spmd(nc, in_maps, core_ids=list(range(NCORES)))
    return np.concatenate([r["out"].T for r in res.results], axis=0)
